# revision 34
# baseline (speedup 1.0000x reference)
"""Trainium2 Bass kernel: image -> additive-sinusoid audio encoding.

Math (per batch image b):
  gray = 255 * (w . rgb);  rev = flip(gray, rows);  avg = mean(gray)
  px   = clip(3*rev - 2*avg, 0, 255)
  A    = where(px==0, 0, exp(ln10 * (px/160 - 1.5)))            # [M=64 rows, N=64 cols]
  y[t] = sum_m A[m, col(t)] * sin(W[m]*t*dt + PHI0[m]),  col(t) = min(t//361, 63)
  audio= clip(0.5 + 2048*y, -32768, 32767)                       # [ns=23152]

Kernel strategy (v2): split image columns even/odd: n = 2*n1 + n2.  With
theta_e/o[i,n1] = W[i]*(2n1 + n2)*361*dt + PHI0[i] and beta[i,r] = W[i]*r*dt,
sin(W t + phi) = sin(theta)cos(beta) + cos(theta)sin(beta), so each batch-half
needs just two K=128 matmuls  out[(b2,n1), r] = [A*sin(theta); A*cos(theta)]^T
@ [2048 cos(beta); 2048 sin(beta)]  (moving bank [128, 409] covers both the
361-sample main blocks and the 48-sample tail of column 63).  clip-after-exp
(min(E, A255)) replaces the pre-exp clip; output is stored as fp16 l/4 (host
rescales + clips), killing the PSUM-prefill matmuls and halving out traffic.
Data-parallel over batch: 8 images per core, partitions = (bh, image-row).
"""

import os

import numpy as np

# ---- problem constants (from the nn.Module definition; input-independent) ----
M = 64
N = 64
FL, FH, FS, T = 80.0, 7600.0, 22050, 1.05
NS = 2 * int(0.5 * FS * T)  # 23152
NUM = NS // N  # 361
RMAX = NS - (N - 1) * NUM  # 409 (last column's sample count)
NPAD = NUM + RMAX  # 770 = padded per-(n1) block in the DRAM output
DT = float(np.float32(1.0 / FS))  # reference rounds dt to f32 (jnp weak typing)
TWO_PI = 2.0 * np.pi
B = 64
N_CORES = 8
B_LOC = B // N_CORES  # 8 images per core
SCALE_SSM = (0.5 / np.sqrt(M)) * 32768.0  # 2048
LN10 = float(np.log(10.0))
EXP_A = LN10 / 160.0
EXP_B = -1.5 * LN10
W0, W1, W2 = 0.2989, 0.5870, 0.1140
C00 = 3.0 * 255.0 * W0  # fold of the 3*255*w0 scale into the gray accumulator
R1 = W1 / W0
R2 = W2 / W0
KAVG2 = 2.0 * 255.0 * W0 / 4096.0  # sum(t) -> 2*avg(gray255) weighting
A255 = float(np.exp(np.float64(EXP_A) * 255.0 + EXP_B))  # A value at px=255
OSC = 0.25  # output stored as (l * OSC) in fp16; host multiplies back


def _make_tables():
    # LCG phase bank (faithful port, ir starts at 0)
    ia, ic, im = 9301, 49297, 233280
    ir = 0
    phi = []
    for _ in range(M):
        ir = (ir * ia + ic) % im
        phi.append(TWO_PI * ir / im)
    phi32 = np.array(phi, np.float64).astype(np.float32)
    w32 = (TWO_PI * FL * (FH / FL) ** (np.arange(M) / (M - 1))).astype(np.float32)

    # fold the row flip (tf.reverse on axis 1) into the tables: row i uses W[63-i]
    wf = w32[::-1].astype(np.float64)
    phif = phi32[::-1].astype(np.float64)

    # theta tables: cols [0:32] even image columns (n=2*n1), [32:64] odd
    n1 = np.arange(32, dtype=np.float64)
    th = np.empty((64, 2, 32), np.float64)
    th[:, 0, :] = wf[:, None] * (2 * n1[None, :] * NUM * DT) + phif[:, None]
    th[:, 1, :] = wf[:, None] * ((2 * n1[None, :] + 1) * NUM * DT) + phif[:, None]
    T_tab = np.empty((128, 64), np.float16)
    T_tab[0:64] = np.sin(th).reshape(64, 64)  # P rows
    T_tab[64:128] = np.cos(th).reshape(64, 64)  # Q rows

    # block-diagonal mean-reduce/broadcast stationary, KAVG2 folded in
    blk = np.zeros((128, 128), np.float16)
    blk[:64, :64] = np.float16(KAVG2)
    blk[64:, 64:] = np.float16(KAVG2)

    # partition-duplication stationary: A2[(pq,i), :] = A[i, :] for both pq
    dup = np.tile(np.eye(64, dtype=np.float16), (2, 2))

    # moving bank [cb; sb]: rows (pq, i), cols r in 0..408
    r_idx = np.arange(RMAX, dtype=np.float64)
    beta = wf[:, None] * (r_idx[None, :] * DT)
    cbsb = np.empty((128, RMAX), np.float16)
    cbsb[0:64] = (SCALE_SSM * np.cos(beta)).astype(np.float16)
    cbsb[64:128] = (SCALE_SSM * np.sin(beta)).astype(np.float16)

    tab = np.concatenate([T_tab, blk, dup, cbsb], axis=1)  # [128, 729]
    return {"tab": np.ascontiguousarray(tab)}


_TABLES = None


def tables():
    global _TABLES
    if _TABLES is None:
        _TABLES = _make_tables()
    return _TABLES


def build_nc():
    import concourse.bacc as bacc
    import concourse.bass as bass
    import concourse.mybir as mybir
    import concourse.tile as tile

    f32 = mybir.dt.float32
    f16 = mybir.dt.float16
    Alu = mybir.AluOpType
    Act = mybir.ActivationFunctionType

    nc = bacc.Bacc(
        "TRN2",
        target_bir_lowering=False,
        debug=False,
        num_devices=N_CORES,
        enable_asserts=False,
        enable_partition_id=False,
    )

    # x pre-packed on host: [p=(bh,i), (c, b2, j)] f16 (halves input traffic;
    # adds ~1e-3 rel err, well under the 2e-2 gate)
    x_d = nc.dram_tensor("x", [128, 768], f16, kind="ExternalInput")
    tab_d = nc.dram_tensor("tab", [128, 729], f16, kind="ExternalInput")
    # audio stored as fp16 l/4, padded: block n1 holds 770 samples (722 real
    # + 48 tail, tail meaningful only for n1=31); host restitches + rescales
    audio_d = nc.dram_tensor("audio", [2, 128, NPAD], f16, kind="ExternalOutput")

    with tile.TileContext(nc) as tc:
        with (
            tc.tile_pool(name="consts", bufs=1) as consts,
            tc.tile_pool(name="work", bufs=1) as work,
            tc.tile_pool(name="outp", bufs=2) as outp,
            tc.tile_pool(name="psum", bufs=1, space=bass.MemorySpace.PSUM) as psum,
        ):
            # ---- input DMAs: x halves on the two HWDGE rings, tables split
            # between sync (small, needed first) and gpsimd SWDGE (cbsb) ----
            # asymmetric 3-way x split: the scalar queue first executes the
            # implicit Exp ACT_TABLE_LOAD (~1.3us) so its ring starts late;
            # SWDGE (gpsimd) has ~1.4us startup but aggregates the
            # DRAM-contiguous rows, so a small chunk lands in time
            X = work.tile([128, 768], f16)
            TAB = consts.tile([128, 729], f16)
            nc.sync.dma_start(out=X[0:96], in_=x_d[0:96], single_packet=True)
            nc.scalar.dma_start(out=X[96:128], in_=x_d[96:128], single_packet=True)
            nc.gpsimd.dma_start(out=TAB[:, 0:320], in_=tab_d[:, 0:320])
            nc.gpsimd.dma_start(out=TAB[:, 320:729], in_=tab_d[:, 320:729])
            Tv = TAB[:, 0:64].rearrange("p (ab n1) -> p ab n1", ab=2)
            blk = TAB[:, 64:192]
            dupT = TAB[:, 192:320]
            cbsb = TAB[:, 320:729]

            expb = consts.tile([128, 1], f32)
            nc.gpsimd.memset(expb[:], EXP_B)

            # ---- PSUM banks (allocated in order => bank alignment) ----
            pA = [psum.tile([128, 512], f32, name=f"pA{i}") for i in range(2)]
            pB = [psum.tile([128, 512], f32, name=f"pB{i}") for i in range(2)]
            A2 = [psum.tile([128, 256], f32, name=f"A2_{i}") for i in range(2)]
            csS2 = psum.tile([128, 4], f32)

            # ---- grayscale: t = R + r1*G + r2*B (255*w0 scale folded later) ----
            Xc = X[:].rearrange("p (c q) -> p c q", c=3)
            t = work.tile([128, 256], f32)
            nc.vector.scalar_tensor_tensor(
                out=t, in0=Xc[:, 1], scalar=float(R1), in1=Xc[:, 0],
                op0=Alu.mult, op1=Alu.add,
            )
            nc.vector.scalar_tensor_tensor(
                out=t, in0=Xc[:, 2], scalar=float(R2), in1=t,
                op0=Alu.mult, op1=Alu.add,
            )
            # per-b2 row sums, KAVG2 folded into blk; DVE accumulates in f32
            # and only rounds the f16 output (verified in sim)
            rs16 = work.tile([128, 4], f16)
            with nc.allow_low_precision(reason="f16 row-sum of bounded grayscale"):
                nc.vector.reduce_sum(
                    out=rs16, in_=t[:].rearrange("p (q j) -> p q j", j=64),
                    axis=mybir.AxisListType.X,
                )
            # one matmul reduces across partitions AND broadcasts: csS2 = 2*avg
            nc.tensor.matmul(csS2, blk, rs16, start=True, stop=True)

            # ---- px = 3*255*w0*t - 2*avg ; A = (px>0) * min(exp(a*px+b), A255) ----
            px = work.tile([128, 256], f32)
            nc.vector.scalar_tensor_tensor(
                out=px[:].rearrange("p (q j) -> p q j", j=64),
                in0=t[:].rearrange("p (q j) -> p q j", j=64),
                scalar=float(C00),
                in1=csS2[:].broadcast_to([128, 4, 64]),
                op0=Alu.mult, op1=Alu.subtract,
            )
            mask = work.tile([128, 256], f32)
            nc.vector.tensor_scalar(
                out=mask, in0=px, scalar1=0.0, scalar2=None, op0=Alu.is_gt
            )
            E = work.tile([128, 256], f32)
            nc.scalar.activation(
                out=E, in_=px, func=Act.Exp, bias=expb[:], scale=float(EXP_A)
            )
            A = work.tile([128, 256], f16)
            nc.vector.scalar_tensor_tensor(
                out=A, in0=E, scalar=float(A255), in1=mask,
                op0=Alu.min, op1=Alu.mult,
            )

            # ---- duplicate A to both partition halves via PE (walrus requires
            # equal base partitions for two-SBUF-input engine ops, so the
            # stationary build reads the aligned PSUM copy instead) ----
            for bh in range(2):
                rows = slice(64 * bh, 64 * bh + 64)
                nc.tensor.matmul(
                    A2[bh], dupT[rows], A[rows], start=True, stop=True
                )

            # ---- stationaries S[bh][ab][k=(pq,i), m=(b2,n1)] = A*theta-bank.
            # Full-width [128, 4, 32] vector ops over the PE-duplicated PSUM
            # copy (engine cost is free-size-bound, so one 128-partition op
            # costs the same as a 64-partition one) ----
            S = [
                [work.tile([128, 128], f16, name=f"S{bh}{ab}") for ab in range(2)]
                for bh in range(2)
            ]
            for bh in range(2):
                A2v = A2[bh][:].rearrange("p (q n1 n2) -> p q n1 n2", n1=32, n2=2)
                for ab in range(2):
                    Sx = S[bh][ab][:].rearrange("p (q n1) -> p q n1", n1=32)
                    nc.vector.tensor_mul(
                        out=Sx, in0=A2v[:, :, :, ab],
                        in1=Tv[:, ab].unsqueeze(1).broadcast_to([128, 4, 32]),
                    )

            # ---- matmuls + converts + output DMAs, pipelined per batch-half ----
            us = []
            for bh in range(2):
                nc.tensor.matmul(
                    pA[bh][:, 0:NUM], S[bh][0], cbsb[:, 0:NUM],
                    start=True, stop=True,
                )
                # bankB covers main (r<361) AND the column-63 tail in one go
                nc.tensor.matmul(
                    pB[bh][:, 0:RMAX], S[bh][1], cbsb[:, 0:RMAX],
                    start=True, stop=True,
                )
                u = outp.tile([128, NPAD], f16, tag="u")
                us.append(u)
                # u = (y + 0.5) / 4 in fp16 (|l|/4 <= ~20k < fp16 max; host
                # multiplies by 4 and applies the final clip). Scalar converts
                # u0's banks, vector u1's, so the last convert chases the last
                # matmul with no queueing behind the other image-half.
                ceng = nc.scalar if bh == 0 else None
                if bh == 0:
                    nc.scalar.activation(
                        out=u[:, 0:NUM], in_=pA[bh][:, 0:NUM],
                        func=Act.Copy, bias=0.5 * OSC, scale=OSC,
                    )
                    nc.scalar.activation(
                        out=u[:, NUM:NPAD], in_=pB[bh][:, 0:RMAX],
                        func=Act.Copy, bias=0.5 * OSC, scale=OSC,
                    )
                else:
                    nc.vector.tensor_scalar(
                        out=u[:, 0:NUM], in0=pA[bh][:, 0:NUM],
                        scalar1=OSC, scalar2=0.5 * OSC, op0=Alu.mult, op1=Alu.add,
                    )
                    nc.vector.tensor_scalar(
                        out=u[:, NUM:NPAD], in0=pB[bh][:, 0:RMAX],
                        scalar1=OSC, scalar2=0.5 * OSC, op0=Alu.mult, op1=Alu.add,
                    )
            nc.sync.dma_start(out=audio_d[0, 0:64], in_=us[0][0:64], single_packet=True)
            nc.scalar.dma_start(out=audio_d[0, 64:128], in_=us[0][64:128], single_packet=True)
            nc.sync.dma_start(out=audio_d[1, 0:64], in_=us[1][0:64], single_packet=True)
            nc.scalar.dma_start(out=audio_d[1, 64:128], in_=us[1][64:128], single_packet=True)

    nc.compile()
    return nc


_NC = None


def _get_nc():
    global _NC
    if _NC is None:
        _NC = build_nc()
    return _NC


LAST_RESULTS = None


def _pack_x(xc: np.ndarray) -> np.ndarray:
    # [8, 64, 64, 3] -> [p=(bh,i), (c, b2, j)] fp16
    return np.ascontiguousarray(
        xc.reshape(2, 4, 64, 64, 3)
        .transpose(0, 2, 4, 1, 3)
        .reshape(128, 768)
        .astype(np.float16)
    )


def _unpack_audio(a: np.ndarray) -> np.ndarray:
    # [2, 128, 770] fp16 (l/4) -> [8, NS] f32
    v = np.clip(a.astype(np.float32) * (1.0 / OSC), -32768.0, 32767.0)
    v = v.reshape(2, 4, 32, NPAD)
    main = v[:, :, :, 0:2 * NUM].reshape(2, 4, 64 * NUM)
    tail = v[:, :, 31, 2 * NUM:NPAD]
    return np.concatenate([main, tail], axis=2).reshape(B_LOC, NS)


def kernel(x: np.ndarray) -> np.ndarray:
    from concourse.bass_utils import run_bass_kernel_spmd

    x = np.ascontiguousarray(np.asarray(x, dtype=np.float32))
    assert x.shape == (B, 64, 64, 3), x.shape

    nc = _get_nc()
    tbl = tables()
    in_maps = []
    for c in range(N_CORES):
        m = {"x": _pack_x(x[c * B_LOC : (c + 1) * B_LOC])}
        m.update(tbl)
        in_maps.append(m)

    trace = os.environ.get("BASS_KERNEL_TRACE", "0") == "1"
    res = run_bass_kernel_spmd(
        nc, in_maps, core_ids=list(range(N_CORES)), trace=trace
    )
    global LAST_RESULTS
    LAST_RESULTS = res
    return np.concatenate([_unpack_audio(r["audio"]) for r in res.results], axis=0)


# revision 35
# speedup vs baseline: 1.0009x; 1.0009x over previous
"""Trainium2 Bass kernel: image -> additive-sinusoid audio encoding.

Math (per batch image b):
  gray = 255 * (w . rgb);  rev = flip(gray, rows);  avg = mean(gray)
  px   = clip(3*rev - 2*avg, 0, 255)
  A    = where(px==0, 0, exp(ln10 * (px/160 - 1.5)))            # [M=64 rows, N=64 cols]
  y[t] = sum_m A[m, col(t)] * sin(W[m]*t*dt + PHI0[m]),  col(t) = min(t//361, 63)
  audio= clip(0.5 + 2048*y, -32768, 32767)                       # [ns=23152]

Kernel strategy (v2): split image columns even/odd: n = 2*n1 + n2.  With
theta_e/o[i,n1] = W[i]*(2n1 + n2)*361*dt + PHI0[i] and beta[i,r] = W[i]*r*dt,
sin(W t + phi) = sin(theta)cos(beta) + cos(theta)sin(beta), so each batch-half
needs just two K=128 matmuls  out[(b2,n1), r] = [A*sin(theta); A*cos(theta)]^T
@ [2048 cos(beta); 2048 sin(beta)]  (moving bank [128, 409] covers both the
361-sample main blocks and the 48-sample tail of column 63).  clip-after-exp
(min(E, A255)) replaces the pre-exp clip; output is stored as fp16 l/4 (host
rescales + clips), killing the PSUM-prefill matmuls and halving out traffic.
Data-parallel over batch: 8 images per core, partitions = (bh, image-row).
"""

import os

import numpy as np

# ---- problem constants (from the nn.Module definition; input-independent) ----
M = 64
N = 64
FL, FH, FS, T = 80.0, 7600.0, 22050, 1.05
NS = 2 * int(0.5 * FS * T)  # 23152
NUM = NS // N  # 361
RMAX = NS - (N - 1) * NUM  # 409 (last column's sample count)
NPAD = NUM + RMAX  # 770 = padded per-(n1) block in the DRAM output
DT = float(np.float32(1.0 / FS))  # reference rounds dt to f32 (jnp weak typing)
TWO_PI = 2.0 * np.pi
B = 64
N_CORES = 8
B_LOC = B // N_CORES  # 8 images per core
SCALE_SSM = (0.5 / np.sqrt(M)) * 32768.0  # 2048
LN10 = float(np.log(10.0))
EXP_A = LN10 / 160.0
EXP_B = -1.5 * LN10
W0, W1, W2 = 0.2989, 0.5870, 0.1140
C00 = 3.0 * 255.0 * W0  # fold of the 3*255*w0 scale into the gray accumulator
R1 = W1 / W0
R2 = W2 / W0
KAVG2 = 2.0 * 255.0 * W0 / 4096.0  # sum(t) -> 2*avg(gray255) weighting
A255 = float(np.exp(np.float64(EXP_A) * 255.0 + EXP_B))  # A value at px=255
OSC = 0.25  # output stored as (l * OSC) in fp16; host multiplies back


def _make_tables():
    # LCG phase bank (faithful port, ir starts at 0)
    ia, ic, im = 9301, 49297, 233280
    ir = 0
    phi = []
    for _ in range(M):
        ir = (ir * ia + ic) % im
        phi.append(TWO_PI * ir / im)
    phi32 = np.array(phi, np.float64).astype(np.float32)
    w32 = (TWO_PI * FL * (FH / FL) ** (np.arange(M) / (M - 1))).astype(np.float32)

    # fold the row flip (tf.reverse on axis 1) into the tables: row i uses W[63-i]
    wf = w32[::-1].astype(np.float64)
    phif = phi32[::-1].astype(np.float64)

    # theta tables: cols [0:32] even image columns (n=2*n1), [32:64] odd
    n1 = np.arange(32, dtype=np.float64)
    th = np.empty((64, 2, 32), np.float64)
    th[:, 0, :] = wf[:, None] * (2 * n1[None, :] * NUM * DT) + phif[:, None]
    th[:, 1, :] = wf[:, None] * ((2 * n1[None, :] + 1) * NUM * DT) + phif[:, None]
    T_tab = np.empty((128, 64), np.float16)
    T_tab[0:64] = np.sin(th).reshape(64, 64)  # P rows
    T_tab[64:128] = np.cos(th).reshape(64, 64)  # Q rows

    # block-diagonal mean-reduce/broadcast stationary, KAVG2 folded in
    blk = np.zeros((128, 128), np.float16)
    blk[:64, :64] = np.float16(KAVG2)
    blk[64:, 64:] = np.float16(KAVG2)

    # partition-duplication stationary: A2[(pq,i), :] = A[i, :] for both pq
    dup = np.tile(np.eye(64, dtype=np.float16), (2, 2))

    # moving bank [cb; sb]: rows (pq, i), cols r in 0..408
    r_idx = np.arange(RMAX, dtype=np.float64)
    beta = wf[:, None] * (r_idx[None, :] * DT)
    cbsb = np.empty((128, RMAX), np.float16)
    cbsb[0:64] = (SCALE_SSM * np.cos(beta)).astype(np.float16)
    cbsb[64:128] = (SCALE_SSM * np.sin(beta)).astype(np.float16)

    tab = np.concatenate([T_tab, blk, dup, cbsb], axis=1)  # [128, 729]
    return {"tab": np.ascontiguousarray(tab)}


_TABLES = None


def tables():
    global _TABLES
    if _TABLES is None:
        _TABLES = _make_tables()
    return _TABLES


def build_nc():
    import concourse.bacc as bacc
    import concourse.bass as bass
    import concourse.mybir as mybir
    import concourse.tile as tile

    f32 = mybir.dt.float32
    f16 = mybir.dt.float16
    Alu = mybir.AluOpType
    Act = mybir.ActivationFunctionType

    nc = bacc.Bacc(
        "TRN2",
        target_bir_lowering=False,
        debug=False,
        num_devices=N_CORES,
        enable_asserts=False,
        enable_partition_id=False,
    )

    # x pre-packed on host: [p=(bh,i), (c, b2, j)] f16 (halves input traffic;
    # adds ~1e-3 rel err, well under the 2e-2 gate)
    x_d = nc.dram_tensor("x", [128, 768], f16, kind="ExternalInput")
    tab_d = nc.dram_tensor("tab", [128, 729], f16, kind="ExternalInput")
    # audio stored as fp16 l/4, padded: block n1 holds 770 samples (722 real
    # + 48 tail, tail meaningful only for n1=31); host restitches + rescales
    audio_d = nc.dram_tensor("audio", [2, 128, NPAD], f16, kind="ExternalOutput")

    with tile.TileContext(nc) as tc:
        with (
            tc.tile_pool(name="consts", bufs=1) as consts,
            tc.tile_pool(name="work", bufs=1) as work,
            tc.tile_pool(name="outp", bufs=2) as outp,
            tc.tile_pool(name="psum", bufs=1, space=bass.MemorySpace.PSUM) as psum,
        ):
            # ---- input DMAs: x halves on the two HWDGE rings, tables split
            # between sync (small, needed first) and gpsimd SWDGE (cbsb) ----
            # asymmetric 3-way x split: the scalar queue first executes the
            # implicit Exp ACT_TABLE_LOAD (~1.3us) so its ring starts late;
            # SWDGE (gpsimd) has ~1.4us startup but aggregates the
            # DRAM-contiguous rows, so a small chunk lands in time
            X = work.tile([128, 768], f16)
            TAB = consts.tile([128, 729], f16)
            nc.sync.dma_start(out=X[0:96], in_=x_d[0:96])
            nc.scalar.dma_start(out=X[96:128], in_=x_d[96:128])
            nc.gpsimd.dma_start(out=TAB[:, 0:320], in_=tab_d[:, 0:320])
            nc.gpsimd.dma_start(out=TAB[:, 320:729], in_=tab_d[:, 320:729])
            Tv = TAB[:, 0:64].rearrange("p (ab n1) -> p ab n1", ab=2)
            blk = TAB[:, 64:192]
            dupT = TAB[:, 192:320]
            cbsb = TAB[:, 320:729]

            expb = consts.tile([128, 1], f32)
            nc.gpsimd.memset(expb[:], EXP_B)

            # ---- PSUM banks (allocated in order => bank alignment) ----
            pA = [psum.tile([128, 512], f32, name=f"pA{i}") for i in range(2)]
            pB = [psum.tile([128, 512], f32, name=f"pB{i}") for i in range(2)]
            A2 = [psum.tile([128, 256], f32, name=f"A2_{i}") for i in range(2)]
            csS2 = psum.tile([128, 4], f32)

            # ---- grayscale: t = R + r1*G + r2*B (255*w0 scale folded later) ----
            Xc = X[:].rearrange("p (c q) -> p c q", c=3)
            t = work.tile([128, 256], f32)
            nc.vector.scalar_tensor_tensor(
                out=t, in0=Xc[:, 1], scalar=float(R1), in1=Xc[:, 0],
                op0=Alu.mult, op1=Alu.add,
            )
            nc.vector.scalar_tensor_tensor(
                out=t, in0=Xc[:, 2], scalar=float(R2), in1=t,
                op0=Alu.mult, op1=Alu.add,
            )
            # per-b2 row sums, KAVG2 folded into blk; DVE accumulates in f32
            # and only rounds the f16 output (verified in sim)
            rs16 = work.tile([128, 4], f16)
            with nc.allow_low_precision(reason="f16 row-sum of bounded grayscale"):
                nc.vector.reduce_sum(
                    out=rs16, in_=t[:].rearrange("p (q j) -> p q j", j=64),
                    axis=mybir.AxisListType.X,
                )
            # one matmul reduces across partitions AND broadcasts: csS2 = 2*avg
            nc.tensor.matmul(csS2, blk, rs16, start=True, stop=True)

            # ---- px = 3*255*w0*t - 2*avg ; A = (px>0) * min(exp(a*px+b), A255) ----
            px = work.tile([128, 256], f32)
            nc.vector.scalar_tensor_tensor(
                out=px[:].rearrange("p (q j) -> p q j", j=64),
                in0=t[:].rearrange("p (q j) -> p q j", j=64),
                scalar=float(C00),
                in1=csS2[:].broadcast_to([128, 4, 64]),
                op0=Alu.mult, op1=Alu.subtract,
            )
            mask = work.tile([128, 256], f32)
            nc.vector.tensor_scalar(
                out=mask, in0=px, scalar1=0.0, scalar2=None, op0=Alu.is_gt
            )
            E = work.tile([128, 256], f32)
            nc.scalar.activation(
                out=E, in_=px, func=Act.Exp, bias=expb[:], scale=float(EXP_A)
            )
            A = work.tile([128, 256], f16)
            nc.vector.scalar_tensor_tensor(
                out=A, in0=E, scalar=float(A255), in1=mask,
                op0=Alu.min, op1=Alu.mult,
            )

            # ---- duplicate A to both partition halves via PE (walrus requires
            # equal base partitions for two-SBUF-input engine ops, so the
            # stationary build reads the aligned PSUM copy instead) ----
            for bh in range(2):
                rows = slice(64 * bh, 64 * bh + 64)
                nc.tensor.matmul(
                    A2[bh], dupT[rows], A[rows], start=True, stop=True
                )

            # ---- stationaries S[bh][ab][k=(pq,i), m=(b2,n1)] = A*theta-bank.
            # Full-width [128, 4, 32] vector ops over the PE-duplicated PSUM
            # copy (engine cost is free-size-bound, so one 128-partition op
            # costs the same as a 64-partition one) ----
            S = [
                [work.tile([128, 128], f16, name=f"S{bh}{ab}") for ab in range(2)]
                for bh in range(2)
            ]
            for bh in range(2):
                A2v = A2[bh][:].rearrange("p (q n1 n2) -> p q n1 n2", n1=32, n2=2)
                for ab in range(2):
                    Sx = S[bh][ab][:].rearrange("p (q n1) -> p q n1", n1=32)
                    nc.vector.tensor_mul(
                        out=Sx, in0=A2v[:, :, :, ab],
                        in1=Tv[:, ab].unsqueeze(1).broadcast_to([128, 4, 32]),
                    )

            # ---- matmuls + converts + output DMAs, pipelined per batch-half ----
            us = []
            for bh in range(2):
                nc.tensor.matmul(
                    pA[bh][:, 0:NUM], S[bh][0], cbsb[:, 0:NUM],
                    start=True, stop=True,
                )
                # bankB covers main (r<361) AND the column-63 tail in one go
                nc.tensor.matmul(
                    pB[bh][:, 0:RMAX], S[bh][1], cbsb[:, 0:RMAX],
                    start=True, stop=True,
                )
                u = outp.tile([128, NPAD], f16, tag="u")
                us.append(u)
                # u = (y + 0.5) / 4 in fp16 (|l|/4 <= ~20k < fp16 max; host
                # multiplies by 4 and applies the final clip). Scalar converts
                # u0's banks, vector u1's, so the last convert chases the last
                # matmul with no queueing behind the other image-half.
                ceng = nc.scalar if bh == 0 else None
                if bh == 0:
                    nc.scalar.activation(
                        out=u[:, 0:NUM], in_=pA[bh][:, 0:NUM],
                        func=Act.Copy, bias=0.5 * OSC, scale=OSC,
                    )
                    nc.scalar.activation(
                        out=u[:, NUM:NPAD], in_=pB[bh][:, 0:RMAX],
                        func=Act.Copy, bias=0.5 * OSC, scale=OSC,
                    )
                else:
                    nc.vector.tensor_scalar(
                        out=u[:, 0:NUM], in0=pA[bh][:, 0:NUM],
                        scalar1=OSC, scalar2=0.5 * OSC, op0=Alu.mult, op1=Alu.add,
                    )
                    nc.vector.tensor_scalar(
                        out=u[:, NUM:NPAD], in0=pB[bh][:, 0:RMAX],
                        scalar1=OSC, scalar2=0.5 * OSC, op0=Alu.mult, op1=Alu.add,
                    )
            nc.sync.dma_start(out=audio_d[0, 0:64], in_=us[0][0:64])
            nc.scalar.dma_start(out=audio_d[0, 64:128], in_=us[0][64:128])
            nc.sync.dma_start(out=audio_d[1, 0:64], in_=us[1][0:64])
            nc.scalar.dma_start(out=audio_d[1, 64:128], in_=us[1][64:128])

    nc.compile()
    return nc


_NC = None


def _get_nc():
    global _NC
    if _NC is None:
        _NC = build_nc()
    return _NC


LAST_RESULTS = None


def _pack_x(xc: np.ndarray) -> np.ndarray:
    # [8, 64, 64, 3] -> [p=(bh,i), (c, b2, j)] fp16
    return np.ascontiguousarray(
        xc.reshape(2, 4, 64, 64, 3)
        .transpose(0, 2, 4, 1, 3)
        .reshape(128, 768)
        .astype(np.float16)
    )


def _unpack_audio(a: np.ndarray) -> np.ndarray:
    # [2, 128, 770] fp16 (l/4) -> [8, NS] f32
    v = np.clip(a.astype(np.float32) * (1.0 / OSC), -32768.0, 32767.0)
    v = v.reshape(2, 4, 32, NPAD)
    main = v[:, :, :, 0:2 * NUM].reshape(2, 4, 64 * NUM)
    tail = v[:, :, 31, 2 * NUM:NPAD]
    return np.concatenate([main, tail], axis=2).reshape(B_LOC, NS)


def kernel(x: np.ndarray) -> np.ndarray:
    from concourse.bass_utils import run_bass_kernel_spmd

    x = np.ascontiguousarray(np.asarray(x, dtype=np.float32))
    assert x.shape == (B, 64, 64, 3), x.shape

    nc = _get_nc()
    tbl = tables()
    in_maps = []
    for c in range(N_CORES):
        m = {"x": _pack_x(x[c * B_LOC : (c + 1) * B_LOC])}
        m.update(tbl)
        in_maps.append(m)

    trace = os.environ.get("BASS_KERNEL_TRACE", "0") == "1"
    res = run_bass_kernel_spmd(
        nc, in_maps, core_ids=list(range(N_CORES)), trace=trace
    )
    global LAST_RESULTS
    LAST_RESULTS = res
    return np.concatenate([_unpack_audio(r["audio"]) for r in res.results], axis=0)
